# revision 10
# baseline (speedup 1.0000x reference)
"""AWQ W4-packed linear layer (y = (x * x_inv_s) @ dequant(w).T + bias)
on 8 Trainium2 NeuronCores, tensor-parallel over out_features (no
collectives).

Per core (1376 of the 11008 out_features):
  * The kernel is tensor-engine bound: 64 token tiles x 32 groups x 1376
    outs of bf16 matmul = 2.82M PE cycles (~1.19 ms at 2.4 GHz). The
    schedule hides everything else under that: compute is emitted as
    three per-oc passes per chunk so the first matmuls start as soon as
    the first third of the weights (WT tile oc0 <- row-tiles 0-3) is
    dequantized, while the rest of weight prep rides under the first
    matmul bursts in the in-order PE queue.
  * Weight prep: wq row-tile DMAs prefetch 2 tiles ahead on the scalar
    queue (DMA triggers block their queue until completion, so that
    queue does nothing else); per half-tile, two wide DVE shift ops
    sign-extend nibbles, 16 fused scalar_tensor_tensor ops apply
    (w*s)*x_inv -> bf16, the PE transposes 8 groups per PSUM bank and a
    wide DVE copy evacuates into the SBUF-resident W^T.
  * Activations: per 512-token chunk, SWDGE cast-DMA f32->bf16 into DRAM
    staging (gpsimd queue), then xbar DMA-transposes on the sync queue
    (one ring; two rings corrupt). Chunks 0/1 are cast in 128-token
    slices so X^T warms up during weight prep.
  * Per (tile, oc): PSUM accumulates 32 matmuls, DVE adds bias, and the
    [128, oc] f32 slice is stored from the sync queue.
"""
import sys

import numpy as np
import ml_dtypes

try:
    import concourse.bass as bass
except ImportError:  # fallback if PYTHONPATH lacks the repo
    for p in ("/opt/trn_rl_repo", "/root/.axon_site/_ro/trn_rl_repo"):
        if p not in sys.path:
            sys.path.append(p)
    import concourse.bass as bass

import concourse.bacc as bacc
import concourse.tile as tile
import concourse.mybir as mybir
from concourse.bass_utils import run_bass_kernel_spmd

F32 = mybir.dt.float32
BF16 = mybir.dt.bfloat16
I32 = mybir.dt.int32

OUT_F, IN_F, GROUP = 11008, 4096, 128
NG = IN_F // GROUP            # 32 quantization groups
NT = 8192                     # B*S tokens
N_CORES = 8
OS = OUT_F // N_CORES         # 1376 out features per core
TCH = 512                     # tokens per chunk
N_CHUNK = NT // TCH           # 16
O_TILES = (OS + 127) // 128   # 11 (last has 96 rows)
OC_SLICES = [(0, 512), (512, 1024), (1024, 1376)]

_NC_CACHE = {}


def build_nc():
    if "nc" in _NC_CACHE:
        return _NC_CACHE["nc"]
    nc = bacc.Bacc("TRN2", target_bir_lowering=False, debug=False,
                   num_devices=N_CORES)

    x = nc.dram_tensor("x", [NT, IN_F], F32, kind="ExternalInput").ap()
    wq = nc.dram_tensor("wq", [OS, NG * 64], I32, kind="ExternalInput").ap()
    ws = nc.dram_tensor("ws", [OS, NG], F32, kind="ExternalInput").ap()
    xinvb = nc.dram_tensor("xinvb", [128, IN_F], BF16,
                           kind="ExternalInput").ap()
    biasb = nc.dram_tensor("biasb", [128, OS], F32, kind="ExternalInput").ap()
    ident = nc.dram_tensor("ident", [128, 128], BF16,
                           kind="ExternalInput").ap()
    y = nc.dram_tensor("y", [NT, OS], F32, kind="ExternalOutput").ap()

    with tile.TileContext(nc) as tc:
        with (
            tc.tile_pool(name="cpool", bufs=1) as cpool,
            tc.tile_pool(name="wpool", bufs=2) as wpool,
            tc.tile_pool(name="xpool", bufs=2) as xpool,
            tc.tile_pool(name="dpool", bufs=2,
                         space=bass.MemorySpace.DRAM) as dpool,
            tc.tile_pool(name="ypool", bufs=4) as ypool,
            tc.tile_pool(name="wps", bufs=4,
                         space=bass.MemorySpace.PSUM) as wps,
            tc.tile_pool(name="aps", bufs=3,
                         space=bass.MemorySpace.PSUM) as aps,
        ):
            # ---------------- persistent tiles ----------------
            WTs = [cpool.tile([128, NG, o1 - o0], BF16, name=f"WT{i}")
                   for i, (o0, o1) in enumerate(OC_SLICES)]   # [i, g, o]
            id_sb = cpool.tile([128, 128], BF16)
            nc.scalar.dma_start(id_sb[:], ident[:])
            xinv_sb = cpool.tile([128, IN_F], BF16)
            nc.scalar.dma_start(xinv_sb[:], xinvb[:])
            bias_sb = cpool.tile([128, OS], F32)

            # ---------------- x pipeline helpers ----------------------
            XTs = {}
            xbss = {}

            def emit_cast(ch, split=1):
                t0 = ch * TCH
                xbs = dpool.tile([TCH, IN_F], BF16, tag="xbs",
                                 name=f"xbs{ch}")
                xbss[ch] = xbs
                step = TCH // split
                for s in range(split):
                    nc.gpsimd.dma_start(
                        xbs[s * step:(s + 1) * step, :],
                        x[t0 + s * step:t0 + (s + 1) * step, :])

            def emit_transposes(ch, split=1):
                xbs = xbss.pop(ch)
                XT = xpool.tile([128, NG, TCH], BF16, tag="xt",
                                name=f"xt{ch}")
                XTs[ch] = XT
                step = TCH // split
                for s in range(split):
                    for g in range(NG):
                        nc.sync.dma_start(
                            XT[:, g, s * step:(s + 1) * step],
                            xbs[s * step:(s + 1) * step,
                                g * 128:(g + 1) * 128],
                            transpose=True)

            # ---------------- weight prep helpers ---------------------
            ws_sbs = {}
            wqts = {}

            def trig(ot):
                r0 = ot * 128
                rows = min(128, OS - r0)
                ws_sb = wpool.tile([128, NG], F32, tag="ws", bufs=4)
                ws_sbs[ot] = ws_sb
                nc.scalar.dma_start(ws_sb[:rows, :], ws[r0:r0 + rows, :])
                for h in range(2):
                    wqt = wpool.tile([128, NG * 32], I32, tag="wq", bufs=4,
                                     name=f"wq{ot}_{h}")
                    wqts[(ot, h)] = wqt
                    nc.scalar.dma_start(
                        wqt[:rows, :],
                        wq[r0:r0 + rows, h * 1024:(h + 1) * 1024])

            def prep(ot):
                if ot + 2 < O_TILES:
                    trig(ot + 2)
                r0 = ot * 128
                rows = min(128, OS - r0)
                oc = min(ot // 4, 2)
                rc = r0 - OC_SLICES[oc][0]
                ws_sb = ws_sbs.pop(ot)
                for h in range(2):          # half-tiles: 16 groups each
                    g0 = h * 16
                    wqt = wqts.pop((ot, h))
                    # sign-extend nibbles: two wide strided DVE ops
                    wint = wpool.tile([128, IN_F // 2], I32, tag="wint",
                                      bufs=1, name=f"wint{ot}_{h}")
                    wr = wint[:rows].rearrange("p (b two) -> p b two", two=2)
                    nc.vector.tensor_scalar(
                        wr[:, :, 0], wqt[:rows, :], 28, 28,
                        mybir.AluOpType.logical_shift_left,
                        mybir.AluOpType.arith_shift_right)
                    nc.vector.tensor_scalar(
                        wr[:, :, 1], wqt[:rows, :], 24, 28,
                        mybir.AluOpType.logical_shift_left,
                        mybir.AluOpType.arith_shift_right)
                    # dequant: (wint * s_g) * xinv -> bf16 on DVE
                    wraw = wpool.tile([128, IN_F // 2], BF16, tag="wraw",
                                      name=f"wraw{ot}_{h}")
                    for j in range(16):
                        g = g0 + j
                        blk = slice(j * 128, (j + 1) * 128)
                        nc.vector.scalar_tensor_tensor(
                            wraw[:rows, blk], wint[:rows, blk],
                            ws_sb[:rows, g:g + 1],
                            xinv_sb[:rows, g * 128:(g + 1) * 128],
                            mybir.AluOpType.mult, mybir.AluOpType.mult)
                    # transpose 8 groups per PSUM bank, wide DVE evac
                    for gb in range(0, 16, 8):
                        pt = wps.tile([128, 8, 128], BF16, tag="pt")
                        for j in range(8):
                            nc.tensor.transpose(
                                pt[:, j, :rows],
                                wraw[:rows, (gb + j) * 128:
                                     (gb + j + 1) * 128],
                                id_sb[:rows, :rows])
                        nc.vector.tensor_scalar(
                            WTs[oc][:, g0 + gb:g0 + gb + 8, rc:rc + rows],
                            pt[:, :, :rows], 1.0, None,
                            mybir.AluOpType.mult)

            # ---------------- compute: one oc pass over a chunk -------
            def emit_compute_oc(ch, oc):
                t0 = ch * TCH
                o0, o1 = OC_SLICES[oc]
                XT = XTs[ch]
                for tt in range(TCH // 128):       # 4 token tiles per chunk
                    trow = t0 + tt * 128
                    acc = aps.tile([128, 512], F32, tag="acc",
                                   name=f"acc{ch}_{tt}_{oc}")
                    for g in range(NG):
                        nc.tensor.matmul(
                            acc[:, :o1 - o0],
                            XT[:, g, tt * 128:(tt + 1) * 128],
                            WTs[oc][:, g, :],
                            start=(g == 0), stop=(g == NG - 1))
                    ystage = ypool.tile([128, 512], F32, tag="ystage",
                                        name=f"ystage{ch}_{tt}_{oc}")
                    nc.vector.tensor_tensor(
                        ystage[:, :o1 - o0], acc[:, :o1 - o0],
                        bias_sb[:, o0:o1], mybir.AluOpType.add)
                    nc.sync.dma_start(y[trow:trow + 128, o0:o1],
                                      ystage[:, :o1 - o0])

            def emit_compute(ch):
                for oc in range(len(OC_SLICES)):
                    emit_compute_oc(ch, oc)
                XTs.pop(ch)

            # ---------------- schedule --------------------------------
            trig(0)
            trig(1)
            # chunk 0 in 128-token slices so XT0 is ready early
            emit_cast(0, split=4)
            emit_transposes(0, split=4)
            prep(0)
            prep(1)
            emit_cast(1, split=2)
            prep(2)
            prep(3)                      # WT[0] complete
            emit_compute_oc(0, 0)
            emit_transposes(1, split=2)
            prep(4)
            prep(5)
            nc.scalar.dma_start(bias_sb[:], biasb[:])
            prep(6)
            prep(7)                      # WT[1] complete
            emit_compute_oc(0, 1)
            emit_cast(2)
            prep(8)
            prep(9)
            prep(10)                     # WT[2] complete
            emit_compute_oc(0, 2)
            XTs.pop(0)
            emit_transposes(2)
            emit_compute(1)
            for ch in range(3, N_CHUNK):
                emit_cast(ch)
                emit_transposes(ch)
                emit_compute(ch - 1)
            emit_compute(N_CHUNK - 1)

    nc.compile()
    _NC_CACHE["nc"] = nc
    return nc


def make_in_maps(x, w_q_packed, w_scales, x_inv_s, bias):
    """Host-side shard + layout prep (reshapes/slices only)."""
    x2 = np.ascontiguousarray(
        np.asarray(x, dtype=np.float32).reshape(NT, IN_F))
    xinvb = np.ascontiguousarray(np.broadcast_to(
        np.asarray(x_inv_s).astype(ml_dtypes.bfloat16), (128, IN_F)))
    ident = np.eye(128, dtype=ml_dtypes.bfloat16)
    wq_full = np.asarray(w_q_packed, dtype=np.int32).reshape(OUT_F, NG * 64)
    ws_full = np.asarray(w_scales, dtype=np.float32).reshape(OUT_F, NG)
    bias_full = np.asarray(bias, dtype=np.float32)
    in_maps = []
    for c in range(N_CORES):
        o0 = c * OS
        in_maps.append({
            "x": x2,
            "wq": np.ascontiguousarray(wq_full[o0:o0 + OS]),
            "ws": np.ascontiguousarray(ws_full[o0:o0 + OS]),
            "xinvb": xinvb,
            "biasb": np.ascontiguousarray(
                np.broadcast_to(bias_full[o0:o0 + OS], (128, OS))
            ).astype(np.float32),
            "ident": ident,
        })
    return in_maps


def kernel(x, w_q_packed, w_scales, x_inv_s, bias):
    """Full inputs in, full output out; shards across 8 cores inside."""
    nc = build_nc()
    in_maps = make_in_maps(x, w_q_packed, w_scales, x_inv_s, bias)
    res = run_bass_kernel_spmd(nc, in_maps, list(range(N_CORES)),
                               trace=False)
    parts = [res.results[c]["y"] for c in range(N_CORES)]
    out = np.concatenate(parts, axis=-1).reshape(4, 2048, OUT_F)
    return out.astype(np.asarray(x).dtype)
